# revision 3
# baseline (speedup 1.0000x reference)
"""Trainium2 Bass kernel for GNN NodeBlock (segment_sum + MLP + layernorm).

Strategy (8 NeuronCores, SPMD single program):
  - Host: assign nodes to cores balancing total degree (serpentine deal by
    degree), then within each core pack nodes into 100 windows of <=128
    nodes each (serpentine again).  Edges are bucketed by the window of
    their receiver, padded to a multiple of 128 per window (shared
    per-window tile counts across cores), and laid out so each 128-edge
    tile lands as [edge_in_tile(partition), feature] in SBUF.
  - Device: per window w, segment-sum via one-hot matmuls:
        onehot[e, s] = (iota[s] == slot[e])          (DVE is_equal)
        aggT[f, s]  += edge_tile[e, f]^T @ onehot    (PE, PSUM accumulate)
    then the node MLP in feature-major layout:
        h1 = relu(W_s_top^T @ x^T + W_s_bot^T @ aggT + b_s)
        h2 = relu(W_h^T @ h1 + b_h)
        out[n, f] = h2^T @ W_e + b_e   (node-major via lhsT=h2)
        layernorm over f (free axis) with bn_stats/bn_aggr.
  - Host: gather per-core outputs back to the global node order.
"""

import sys
import os

if "/opt/trn_rl_repo" not in sys.path:
    sys.path.insert(0, "/opt/trn_rl_repo")

import numpy as np

N_NODES = 100000
N_EDGES = 1600000
D = 128
N_CORES = 8
NW = 100                      # windows (128-node groups) per core
NODES_PER_CORE = NW * 128     # local node-table slots per core
EPS = 1e-5


# ----------------------------------------------------------------------------
# Host-side packing
# ----------------------------------------------------------------------------

def _serpentine(n, k):
    """Positions 0..n-1 dealt serpentine over k bins; returns (bin, rank)."""
    i = np.arange(n)
    block = i // k
    pos = i % k
    b = np.where(block % 2 == 0, pos, k - 1 - pos)
    return b, block


def _pack(x, edge_attr, receivers):
    rec = np.asarray(receivers).astype(np.int64)
    x = np.asarray(x, dtype=np.float32)
    ea = np.ascontiguousarray(np.asarray(edge_attr, dtype=np.float32))

    deg = np.bincount(rec, minlength=N_NODES)

    # nodes -> cores (serpentine by degree desc => per-core degree sums equal)
    order = np.argsort(-deg, kind="stable")
    core_of_rank, _ = _serpentine(N_NODES, N_CORES)
    node_core = np.empty(N_NODES, np.int64)
    node_core[order] = core_of_rank

    # nodes -> (window, slot) within each core, serpentine by degree desc
    node_win = np.empty(N_NODES, np.int64)
    node_slot = np.empty(N_NODES, np.int64)
    for c in range(N_CORES):
        nodes_c = order[core_of_rank == c]       # degree-desc order
        w, s = _serpentine(len(nodes_c), NW)
        assert s.max() < 128
        node_win[nodes_c] = w
        node_slot[nodes_c] = s

    # per-(core, window) edge counts -> shared tile counts per window
    erec_core = node_core[rec]
    erec_win = node_win[rec]
    ekey = erec_core * NW + erec_win
    Lcw = np.bincount(ekey, minlength=N_CORES * NW).reshape(N_CORES, NW)
    tiles = np.maximum(1, -(-Lcw.max(axis=0) // 128))      # [NW]
    t_total = int(tiles.sum())
    rows = t_total * 128

    # padded row position of every edge inside its core's edge block
    eorder = np.argsort(ekey, kind="stable")
    gsizes = Lcw.reshape(-1)
    gstart = np.concatenate([[0], np.cumsum(gsizes)])[:-1]
    rank_in_group = np.arange(N_EDGES) - gstart[ekey[eorder]]
    row_base = np.concatenate([[0], np.cumsum(tiles * 128)])[:-1]   # per window
    pad_row = row_base[erec_win[eorder]] + rank_in_group
    core_o = erec_core[eorder]

    # build per-core padded edge blocks + slot table
    edges_rows = np.zeros((N_CORES, rows, D), np.float32)
    edges_rows[core_o, pad_row] = ea[eorder]
    slot_rows = np.zeros((N_CORES, rows), np.float32)
    slot_rows[core_o, pad_row] = node_slot[rec][eorder].astype(np.float32)

    # device layout: [128 partitions, t_total*128] with tile t at cols
    # [t*128, (t+1)*128) and partition p = edge (t*128+p)'s features.
    edges_dev = np.ascontiguousarray(
        edges_rows.reshape(N_CORES, t_total, 128, D)
        .transpose(0, 2, 1, 3)
        .reshape(N_CORES, 128, t_total * D)
    )
    rrel_dev = np.ascontiguousarray(
        slot_rows.reshape(N_CORES, t_total, 128).transpose(0, 2, 1)
    )  # [cores, 128, t_total]

    # per-core x table in local node order
    x_loc = np.zeros((N_CORES, NODES_PER_CORE, D), np.float32)
    nloc = node_win * 128 + node_slot
    x_loc[node_core, nloc] = x

    return dict(
        tiles=tuple(int(t) for t in tiles),
        edges_dev=edges_dev,
        rrel_dev=rrel_dev,
        x_loc=x_loc,
        node_core=node_core,
        nloc=nloc,
    )


# ----------------------------------------------------------------------------
# Device program
# ----------------------------------------------------------------------------

_KERNEL_CACHE = {}


def _build(tiles):
    import concourse.bass as bass
    import concourse.tile as tile
    from concourse import bacc, mybir

    f32 = mybir.dt.float32
    t_total = sum(tiles)

    nc = bacc.Bacc("TRN2", target_bir_lowering=False, debug=False,
                   num_devices=N_CORES)

    edges_d = nc.dram_tensor("edges", [128, t_total * D], f32, kind="ExternalInput")
    rrel_d = nc.dram_tensor("rrel", [128, t_total], f32, kind="ExternalInput")
    x_d = nc.dram_tensor("x_loc", [NODES_PER_CORE, D], f32, kind="ExternalInput")
    iota_d = nc.dram_tensor("iota", [128, 128], f32, kind="ExternalInput")
    ident_d = nc.dram_tensor("ident", [128, 128], f32, kind="ExternalInput")
    wst_d = nc.dram_tensor("w_s_top", [128, 128], f32, kind="ExternalInput")
    wsb_d = nc.dram_tensor("w_s_bot", [128, 128], f32, kind="ExternalInput")
    wh_d = nc.dram_tensor("w_h", [128, 128], f32, kind="ExternalInput")
    we_d = nc.dram_tensor("w_e", [128, 128], f32, kind="ExternalInput")
    bs_d = nc.dram_tensor("b_s", [128, 1], f32, kind="ExternalInput")
    bh_d = nc.dram_tensor("b_h", [128, 1], f32, kind="ExternalInput")
    be_d = nc.dram_tensor("b_e_bcast", [128, 128], f32, kind="ExternalInput")
    out_d = nc.dram_tensor("out", [NODES_PER_CORE, D], f32, kind="ExternalOutput")

    Relu = mybir.ActivationFunctionType.Relu
    Ident = mybir.ActivationFunctionType.Identity
    Sqrt = mybir.ActivationFunctionType.Sqrt
    EQ = mybir.AluOpType.is_equal

    with tile.TileContext(nc) as tc:
        with (
            tc.tile_pool(name="const", bufs=1) as constp,
            tc.tile_pool(name="edges", bufs=3) as edgep,
            tc.tile_pool(name="oh", bufs=4) as ohp,
            tc.tile_pool(name="xn", bufs=3) as xnp,
            tc.tile_pool(name="acts", bufs=3) as actp,
            tc.tile_pool(name="small", bufs=4) as smallp,
            tc.tile_pool(name="outs", bufs=3) as outp,
            tc.tile_pool(name="psA", bufs=2, space="PSUM") as psA,
            tc.tile_pool(name="psX", bufs=2, space="PSUM") as psX,
            tc.tile_pool(name="psM", bufs=4, space="PSUM") as psM,
        ):
            # constants / weights
            iota = constp.tile([128, 128], f32)
            nc.gpsimd.dma_start(iota[:], iota_d[:])
            ident = constp.tile([128, 128], f32)
            nc.gpsimd.dma_start(ident[:], ident_d[:])
            wst = constp.tile([128, 128], f32)
            nc.gpsimd.dma_start(wst[:], wst_d[:])
            wsb = constp.tile([128, 128], f32)
            nc.gpsimd.dma_start(wsb[:], wsb_d[:])
            wh = constp.tile([128, 128], f32)
            nc.gpsimd.dma_start(wh[:], wh_d[:])
            we = constp.tile([128, 128], f32)
            nc.gpsimd.dma_start(we[:], we_d[:])
            bs = constp.tile([128, 1], f32)
            nc.gpsimd.dma_start(bs[:], bs_d[:])
            bh = constp.tile([128, 1], f32)
            nc.gpsimd.dma_start(bh[:], bh_d[:])
            be = constp.tile([128, 128], f32)
            nc.gpsimd.dma_start(be[:], be_d[:])
            rrel = constp.tile([128, t_total], f32)
            nc.gpsimd.dma_start(rrel[:], rrel_d[:])

            col = 0
            for w in range(NW):
                T = tiles[w]
                c0 = col * 128

                ew = edgep.tile([128, T * 128], f32, tag="ew")
                nc.gpsimd.dma_start(ew[:], edges_d[:, c0:c0 + T * 128])

                ps_agg = psA.tile([128, 128], f32)
                for t in range(T):
                    oh = ohp.tile([128, 128], f32, tag="oh")
                    nc.vector.tensor_scalar(
                        oh[:], iota[:], rrel[:, col + t:col + t + 1], None, EQ)
                    nc.tensor.matmul(
                        ps_agg[:], ew[:, t * 128:(t + 1) * 128], oh[:],
                        start=(t == 0), stop=(t == T - 1))
                agg = actp.tile([128, 128], f32, tag="agg")
                nc.scalar.copy(agg[:], ps_agg[:])

                # x tile -> transpose to feature-major
                xn = xnp.tile([128, 128], f32, tag="xn")
                nc.gpsimd.dma_start(xn[:], x_d[w * 128:(w + 1) * 128, :])
                ps_xt = psX.tile([128, 128], f32)
                nc.tensor.transpose(ps_xt[:], xn[:], ident[:])
                xt = xnp.tile([128, 128], f32, tag="xt")
                nc.scalar.copy(xt[:], ps_xt[:])

                # MLP layer 1: h1 = relu(Wst^T xT + Wsb^T aggT + b_s)
                ps_h = psM.tile([128, 128], f32, tag="mlp")
                nc.tensor.matmul(ps_h[:], wst[:], xt[:], start=True, stop=False)
                nc.tensor.matmul(ps_h[:], wsb[:], agg[:], start=False, stop=True)
                h1 = actp.tile([128, 128], f32, tag="h1")
                nc.scalar.activation(h1[:], ps_h[:], Relu, bias=bs[:, 0:1])

                # layer 2
                ps_h2 = psM.tile([128, 128], f32, tag="mlp")
                nc.tensor.matmul(ps_h2[:], wh[:], h1[:], start=True, stop=True)
                h2 = actp.tile([128, 128], f32, tag="h2")
                nc.scalar.activation(h2[:], ps_h2[:], Relu, bias=bh[:, 0:1])

                # output layer, node-major: out[n, f] = h2^T @ W_e
                ps_o = psM.tile([128, 128], f32, tag="mlp")
                nc.tensor.matmul(ps_o[:], h2[:], we[:], start=True, stop=True)
                v = outp.tile([128, 128], f32, tag="v")
                nc.vector.tensor_add(v[:], ps_o[:], be[:])

                # layernorm over free axis
                st = smallp.tile([128, 6], f32, tag="st")
                nc.vector.bn_stats(st[:], v[:])
                mv = smallp.tile([128, 2], f32, tag="mv")
                nc.vector.bn_aggr(mv[:], st[:])
                ve = smallp.tile([128, 1], f32, tag="ve")
                nc.vector.tensor_scalar_add(ve[:], mv[:, 1:2], EPS)
                sd = smallp.tile([128, 1], f32, tag="sd")
                nc.scalar.activation(sd[:], ve[:], Sqrt, bias=0.0)
                istd = smallp.tile([128, 1], f32, tag="istd")
                nc.vector.reciprocal(istd[:], sd[:])
                nm = smallp.tile([128, 1], f32, tag="nm")
                nc.vector.tensor_mul(nm[:], mv[:, 0:1], istd[:])
                nm2 = smallp.tile([128, 1], f32, tag="nm2")
                nc.vector.tensor_scalar_mul(nm2[:], nm[:], -1.0)

                o = outp.tile([128, 128], f32, tag="o")
                nc.scalar.activation(o[:], v[:], Ident, bias=nm2[:, 0:1],
                                     scale=istd[:, 0:1])
                nc.gpsimd.dma_start(out_d[w * 128:(w + 1) * 128, :], o[:])

                col += T

    nc.compile()
    return nc


def _get_kernel(tiles):
    key = tuple(tiles)
    if key not in _KERNEL_CACHE:
        _KERNEL_CACHE[key] = _build(key)
    return _KERNEL_CACHE[key]


# ----------------------------------------------------------------------------
# Entry point
# ----------------------------------------------------------------------------

def kernel(x, edge_attr, receivers, senders, W_s, b_s, W_h, b_h, W_e, b_e):
    from concourse.bass_utils import run_bass_kernel_spmd

    x = np.asarray(x, dtype=np.float32)
    edge_attr_np = np.asarray(edge_attr)
    W_s = np.asarray(W_s, dtype=np.float32)
    b_s = np.asarray(b_s, dtype=np.float32)
    W_h = np.asarray(W_h, dtype=np.float32)
    b_h = np.asarray(b_h, dtype=np.float32)
    W_e = np.asarray(W_e, dtype=np.float32)
    b_e = np.asarray(b_e, dtype=np.float32)

    p = _pack(x, edge_attr_np, receivers)
    nc = _get_kernel(p["tiles"])

    iota = np.tile(np.arange(128, dtype=np.float32), (128, 1))
    ident = np.eye(128, dtype=np.float32)
    common = {
        "iota": iota,
        "ident": ident,
        "w_s_top": np.ascontiguousarray(W_s[:128, :]),
        "w_s_bot": np.ascontiguousarray(W_s[128:, :]),
        "w_h": W_h,
        "w_e": W_e,
        "b_s": b_s.reshape(128, 1),
        "b_h": b_h.reshape(128, 1),
        "b_e_bcast": np.tile(b_e.reshape(1, 128), (128, 1)),
    }
    in_maps = []
    for c in range(N_CORES):
        m = dict(common)
        m["edges"] = p["edges_dev"][c]
        m["rrel"] = p["rrel_dev"][c]
        m["x_loc"] = p["x_loc"][c]
        in_maps.append(m)

    res = run_bass_kernel_spmd(nc, in_maps, list(range(N_CORES)))

    out_full = np.empty((N_NODES, D), np.float32)
    node_core = p["node_core"]
    nloc = p["nloc"]
    for c in range(N_CORES):
        mask = node_core == c
        out_full[mask] = res.results[c]["out"][nloc[mask]]

    return (out_full, edge_attr_np, np.asarray(receivers), np.asarray(senders))


# revision 5
# speedup vs baseline: 1.3007x; 1.3007x over previous
"""Trainium2 Bass kernel for GNN NodeBlock (segment_sum + MLP + layernorm).

Strategy (8 NeuronCores, SPMD single program):
  - Host: deal nodes to cores serpentine by degree (equal per-core edge
    counts), then within each core sort nodes by degree desc and cut into
    98 windows of 128 nodes ("slots").  Edges are laid out SLOT-ALIGNED:
    tile t of window w holds the t-th edge of every slot (one edge per
    partition), zero-padded.  Grouping equal-degree nodes per window makes
    the padding ~1.3%.
  - Device segment-sum needs NO one-hot and NO DVE work: for each tile,
        matmul(psum, lhsT=edge_tile[s, f], rhs=identity)  accumulates
        aggT[f, s] += edge_tile[s, f]^T
    into PSUM (feature-major window aggregate).
  - MLP in feature-major layout (x pre-transposed on host):
        h1 = relu(W_s_top^T @ x^T + W_s_bot^T @ aggT + b_s)
        h2 = relu(W_h^T @ h1 + b_h)
        out[n, f] = h2^T @ W_e + b_e      (node-major via lhsT=h2)
    then layernorm over f (free axis) with bn_stats/bn_aggr in fp32.
  - Segment-sum + MLP run in bf16 (fp32 PSUM accumulate); layernorm fp32.
  - Host: gather per-core outputs back to the global node order.
"""

import sys

if "/opt/trn_rl_repo" not in sys.path:
    sys.path.insert(0, "/opt/trn_rl_repo")

import numpy as np
import ml_dtypes

N_NODES = 100000
N_EDGES = 1600000
D = 128
N_CORES = 8
NW = 98                       # windows (128-node groups) per core
NODES_PER_CORE = NW * 128     # local node-table slots per core
EPS = 1e-5
BF16 = True                   # compute dtype for segment-sum + MLP


# ----------------------------------------------------------------------------
# Host-side packing
# ----------------------------------------------------------------------------

def _pack(x, edge_attr, receivers):
    rec = np.asarray(receivers).astype(np.int64)
    x = np.asarray(x, dtype=np.float32)
    ea = np.ascontiguousarray(np.asarray(edge_attr, dtype=np.float32))

    deg = np.bincount(rec, minlength=N_NODES)

    # nodes -> cores, serpentine by degree desc => equal per-core edge counts
    order = np.argsort(-deg, kind="stable")
    i = np.arange(N_NODES)
    blk, pos = i // N_CORES, i % N_CORES
    core_of_rank = np.where(blk % 2 == 0, pos, N_CORES - 1 - pos)
    node_core = np.empty(N_NODES, np.int64)
    node_core[order] = core_of_rank

    # within each core: degree-desc blocks of 128 = windows; rank in block = slot
    node_win = np.empty(N_NODES, np.int64)
    node_slot = np.empty(N_NODES, np.int64)
    deg_sorted_by_core = {}
    for c in range(N_CORES):
        nodes_c = order[core_of_rank == c]          # degree-desc
        j = np.arange(len(nodes_c))
        node_win[nodes_c] = j // 128
        node_slot[nodes_c] = j % 128
        deg_sorted_by_core[c] = deg[nodes_c]

    # shared per-window tile counts = max degree in window across cores
    tiles = np.zeros(NW, np.int64)
    for c in range(N_CORES):
        dc = deg_sorted_by_core[c]
        for w in range(NW):
            seg = dc[w * 128:(w + 1) * 128]
            if len(seg):
                tiles[w] = max(tiles[w], int(seg.max()))
    tiles = np.maximum(tiles, 1)
    t_total = int(tiles.sum())
    rows = t_total * 128

    # edge row position: window base + j*128 + slot, j = index among node's edges
    erec = rec
    eorder = np.argsort(erec, kind="stable")        # group edges by node
    starts = np.concatenate([[0], np.cumsum(deg)])[:-1]
    j_of = np.arange(N_EDGES) - starts[erec[eorder]]

    row_base = np.concatenate([[0], np.cumsum(tiles * 128)])[:-1]   # per window
    n_of = erec[eorder]
    pad_row = row_base[node_win[n_of]] + j_of * 128 + node_slot[n_of]
    core_o = node_core[n_of]

    dt = ml_dtypes.bfloat16 if BF16 else np.float32
    edges_rows = np.zeros((N_CORES, rows, D), dt)
    edges_rows[core_o, pad_row] = ea[eorder].astype(dt)

    # device layout: [128 partitions(=slot), t_total*128] tile-major
    edges_dev = np.ascontiguousarray(
        edges_rows.reshape(N_CORES, t_total, 128, D)
        .transpose(0, 2, 1, 3)
        .reshape(N_CORES, 128, t_total * D)
    )

    # per-core transposed x table [128(feat), NODES_PER_CORE] in local order
    x_loc = np.zeros((N_CORES, NODES_PER_CORE, D), np.float32)
    nloc = node_win * 128 + node_slot
    x_loc[node_core, nloc] = x
    xt_loc = np.ascontiguousarray(x_loc.transpose(0, 2, 1).astype(dt))

    return dict(
        tiles=tuple(int(t) for t in tiles),
        edges_dev=edges_dev,
        xt_loc=xt_loc,
        node_core=node_core,
        nloc=nloc,
    )


# ----------------------------------------------------------------------------
# Device program
# ----------------------------------------------------------------------------

_KERNEL_CACHE = {}


def _build(tiles):
    import concourse.bass as bass
    import concourse.tile as tile
    from concourse import bacc, mybir

    f32 = mybir.dt.float32
    cdt = mybir.dt.bfloat16 if BF16 else mybir.dt.float32
    t_total = sum(tiles)

    nc = bacc.Bacc("TRN2", target_bir_lowering=False, debug=False,
                   num_devices=N_CORES)

    edges_d = nc.dram_tensor("edges", [128, t_total * D], cdt, kind="ExternalInput")
    xt_d = nc.dram_tensor("xt_loc", [128, NODES_PER_CORE], cdt, kind="ExternalInput")
    ident_d = nc.dram_tensor("ident", [128, 128], cdt, kind="ExternalInput")
    wst_d = nc.dram_tensor("w_s_top", [128, 128], cdt, kind="ExternalInput")
    wsb_d = nc.dram_tensor("w_s_bot", [128, 128], cdt, kind="ExternalInput")
    wh_d = nc.dram_tensor("w_h", [128, 128], cdt, kind="ExternalInput")
    we_d = nc.dram_tensor("w_e", [128, 128], cdt, kind="ExternalInput")
    bs_d = nc.dram_tensor("b_s", [128, 1], f32, kind="ExternalInput")
    bh_d = nc.dram_tensor("b_h", [128, 1], f32, kind="ExternalInput")
    be_d = nc.dram_tensor("b_e_bcast", [128, 128], f32, kind="ExternalInput")
    out_d = nc.dram_tensor("out", [NODES_PER_CORE, D], f32, kind="ExternalOutput")

    Relu = mybir.ActivationFunctionType.Relu
    Sqrt = mybir.ActivationFunctionType.Sqrt
    MUL = mybir.AluOpType.mult
    ADD = mybir.AluOpType.add

    with tile.TileContext(nc) as tc:
        with (
            tc.tile_pool(name="const", bufs=1) as constp,
            tc.tile_pool(name="edges", bufs=3) as edgep,
            tc.tile_pool(name="acts", bufs=3) as actp,
            tc.tile_pool(name="small", bufs=4) as smallp,
            tc.tile_pool(name="outs", bufs=3) as outp,
            tc.tile_pool(name="psA", bufs=3, space="PSUM") as psA,
            tc.tile_pool(name="psM", bufs=4, space="PSUM") as psM,
        ):
            ident = constp.tile([128, 128], cdt)
            nc.scalar.dma_start(ident[:], ident_d[:])
            wst = constp.tile([128, 128], cdt)
            nc.scalar.dma_start(wst[:], wst_d[:])
            wsb = constp.tile([128, 128], cdt)
            nc.scalar.dma_start(wsb[:], wsb_d[:])
            wh = constp.tile([128, 128], cdt)
            nc.scalar.dma_start(wh[:], wh_d[:])
            we = constp.tile([128, 128], cdt)
            nc.scalar.dma_start(we[:], we_d[:])
            bs = constp.tile([128, 1], f32)
            nc.scalar.dma_start(bs[:], bs_d[:])
            bh = constp.tile([128, 1], f32)
            nc.scalar.dma_start(bh[:], bh_d[:])
            be = constp.tile([128, 128], f32)
            nc.scalar.dma_start(be[:], be_d[:])
            xt = constp.tile([128, NODES_PER_CORE], cdt)
            nc.scalar.dma_start(xt[:], xt_d[:])

            col = 0
            for w in range(NW):
                T = tiles[w]
                c0 = col * 128

                ew = edgep.tile([128, T * 128], cdt, tag="ew")
                nc.scalar.dma_start(ew[:], edges_d[:, c0:c0 + T * 128])

                # segment sum: aggT[f, s] += edge_tile[s, f]^T  (rhs = identity)
                ps_agg = psA.tile([128, 128], f32)
                for t in range(T):
                    nc.tensor.matmul(
                        ps_agg[:], ew[:, t * 128:(t + 1) * 128], ident[:],
                        start=(t == 0), stop=(t == T - 1))
                agg = actp.tile([128, 128], cdt, tag="agg")
                nc.scalar.copy(agg[:], ps_agg[:])

                # MLP layer 1
                ps_h = psM.tile([128, 128], f32, tag="mlp")
                nc.tensor.matmul(ps_h[:], wst[:], xt[:, w * 128:(w + 1) * 128],
                                 start=True, stop=False)
                nc.tensor.matmul(ps_h[:], wsb[:], agg[:], start=False, stop=True)
                h1 = actp.tile([128, 128], cdt, tag="h1")
                nc.scalar.activation(h1[:], ps_h[:], Relu, bias=bs[:, 0:1])

                # layer 2
                ps_h2 = psM.tile([128, 128], f32, tag="mlp")
                nc.tensor.matmul(ps_h2[:], wh[:], h1[:], start=True, stop=True)
                h2 = actp.tile([128, 128], cdt, tag="h2")
                nc.scalar.activation(h2[:], ps_h2[:], Relu, bias=bh[:, 0:1])

                # output layer, node-major
                ps_o = psM.tile([128, 128], f32, tag="mlp")
                nc.tensor.matmul(ps_o[:], h2[:], we[:], start=True, stop=True)
                v = outp.tile([128, 128], f32, tag="v")
                nc.vector.tensor_add(v[:], ps_o[:], be[:])

                # layernorm over free axis (fp32)
                st = smallp.tile([128, 6], f32, tag="st")
                nc.vector.bn_stats(st[:], v[:])
                mv = smallp.tile([128, 2], f32, tag="mv")
                nc.vector.bn_aggr(mv[:], st[:])
                ve = smallp.tile([128, 1], f32, tag="ve")
                nc.vector.tensor_scalar_add(ve[:], mv[:, 1:2], EPS)
                sd = smallp.tile([128, 1], f32, tag="sd")
                nc.scalar.activation(sd[:], ve[:], Sqrt, bias=0.0)
                istd = smallp.tile([128, 1], f32, tag="istd")
                nc.vector.reciprocal(istd[:], sd[:])
                nm = smallp.tile([128, 1], f32, tag="nm")
                nc.vector.tensor_scalar(nm[:], mv[:, 0:1], -1.0, istd[:, 0:1],
                                        MUL, MUL)
                o = outp.tile([128, 128], f32, tag="o")
                nc.vector.tensor_scalar(o[:], v[:], istd[:, 0:1], nm[:, 0:1],
                                        MUL, ADD)
                nc.scalar.dma_start(out_d[w * 128:(w + 1) * 128, :], o[:])

                col += T

    nc.compile()
    return nc


def _get_kernel(tiles):
    key = tuple(tiles)
    if key not in _KERNEL_CACHE:
        _KERNEL_CACHE[key] = _build(key)
    return _KERNEL_CACHE[key]


# ----------------------------------------------------------------------------
# Entry point
# ----------------------------------------------------------------------------

def kernel(x, edge_attr, receivers, senders, W_s, b_s, W_h, b_h, W_e, b_e):
    from concourse.bass_utils import run_bass_kernel_spmd

    x = np.asarray(x, dtype=np.float32)
    edge_attr_np = np.asarray(edge_attr)
    W_s = np.asarray(W_s, dtype=np.float32)
    b_s = np.asarray(b_s, dtype=np.float32)
    W_h = np.asarray(W_h, dtype=np.float32)
    b_h = np.asarray(b_h, dtype=np.float32)
    W_e = np.asarray(W_e, dtype=np.float32)
    b_e = np.asarray(b_e, dtype=np.float32)

    p = _pack(x, edge_attr_np, receivers)
    nc = _get_kernel(p["tiles"])

    dt = ml_dtypes.bfloat16 if BF16 else np.float32
    common = {
        "ident": np.eye(128, dtype=dt),
        "w_s_top": np.ascontiguousarray(W_s[:128, :]).astype(dt),
        "w_s_bot": np.ascontiguousarray(W_s[128:, :]).astype(dt),
        "w_h": W_h.astype(dt),
        "w_e": W_e.astype(dt),
        "b_s": b_s.reshape(128, 1),
        "b_h": b_h.reshape(128, 1),
        "b_e_bcast": np.tile(b_e.reshape(1, 128), (128, 1)),
    }
    in_maps = []
    for c in range(N_CORES):
        m = dict(common)
        m["edges"] = p["edges_dev"][c]
        m["xt_loc"] = p["xt_loc"][c]
        in_maps.append(m)

    res = run_bass_kernel_spmd(nc, in_maps, list(range(N_CORES)))

    out_full = np.empty((N_NODES, D), np.float32)
    node_core = p["node_core"]
    nloc = p["nloc"]
    for c in range(N_CORES):
        mask = node_core == c
        out_full[mask] = res.results[c]["out"][nloc[mask]]

    return (out_full, edge_attr_np, np.asarray(receivers), np.asarray(senders))


# revision 12
# speedup vs baseline: 1.5544x; 1.1950x over previous
"""Trainium2 Bass kernel for GNN NodeBlock (segment_sum + MLP + layernorm).

Strategy (8 NeuronCores, SPMD single program):
  - Host: deal nodes to cores serpentine by degree (equal per-core edge
    counts), then within each core sort nodes by degree desc and cut into
    98 windows of 128 nodes ("slots").  Edges are laid out SLOT-ALIGNED:
    tile t of window w holds the t-th edge of every slot (one edge per
    partition), zero-padded.  Grouping equal-degree nodes per window makes
    the padding ~1.3%.
  - Device segment-sum needs NO one-hot and NO DVE work: for each tile,
        matmul(psum, lhsT=edge_tile[s, f], rhs=identity)  accumulates
        aggT[f, s] += edge_tile[s, f]^T
    into PSUM (feature-major window aggregate).
  - MLP in feature-major layout (x pre-transposed on host):
        h1 = relu(W_s_top^T @ x^T + W_s_bot^T @ aggT + b_s)
        h2 = relu(W_h^T @ h1 + b_h)
        out[n, f] = h2^T @ W_e + b_e      (node-major via lhsT=h2)
    then layernorm over f (free axis) with bn_stats/bn_aggr in fp32.
  - Segment-sum + MLP run in bf16 (fp32 PSUM accumulate); layernorm fp32.
  - Host: gather per-core outputs back to the global node order.
"""

import sys

if "/opt/trn_rl_repo" not in sys.path:
    sys.path.insert(0, "/opt/trn_rl_repo")

import numpy as np
import ml_dtypes

N_NODES = 100000
N_EDGES = 1600000
D = 128
N_CORES = 8
NW = 98                       # windows (128-node groups) per core
NODES_PER_CORE = NW * 128     # local node-table slots per core
EPS = 1e-5
BF16 = True                   # compute dtype for segment-sum + MLP


# ----------------------------------------------------------------------------
# Host-side packing
# ----------------------------------------------------------------------------

def _pack(x, edge_attr, receivers):
    rec = np.asarray(receivers).astype(np.int64)
    x = np.asarray(x, dtype=np.float32)
    ea = np.ascontiguousarray(np.asarray(edge_attr, dtype=np.float32))

    deg = np.bincount(rec, minlength=N_NODES)

    # nodes -> cores, serpentine by degree desc => equal per-core edge counts
    order = np.argsort(-deg, kind="stable")
    i = np.arange(N_NODES)
    blk, pos = i // N_CORES, i % N_CORES
    core_of_rank = np.where(blk % 2 == 0, pos, N_CORES - 1 - pos)
    node_core = np.empty(N_NODES, np.int64)
    node_core[order] = core_of_rank

    # within each core: degree-desc blocks of 128 = windows; rank in block = slot
    node_win = np.empty(N_NODES, np.int64)
    node_slot = np.empty(N_NODES, np.int64)
    deg_sorted_by_core = {}
    for c in range(N_CORES):
        nodes_c = order[core_of_rank == c]          # degree-desc
        j = np.arange(len(nodes_c))
        node_win[nodes_c] = j // 128
        node_slot[nodes_c] = j % 128
        deg_sorted_by_core[c] = deg[nodes_c]

    # shared per-window tile counts = max degree in window across cores
    tiles = np.zeros(NW, np.int64)
    for c in range(N_CORES):
        dc = deg_sorted_by_core[c]
        for w in range(NW):
            seg = dc[w * 128:(w + 1) * 128]
            if len(seg):
                tiles[w] = max(tiles[w], int(seg.max()))
    tiles = np.maximum(tiles, 1)
    t_total = int(tiles.sum())
    rows = t_total * 128

    # edge row position: window base + j*128 + slot, j = index among node's edges
    erec = rec
    eorder = np.argsort(erec, kind="stable")        # group edges by node
    starts = np.concatenate([[0], np.cumsum(deg)])[:-1]
    j_of = np.arange(N_EDGES) - starts[erec[eorder]]

    row_base = np.concatenate([[0], np.cumsum(tiles * 128)])[:-1]   # per window
    n_of = erec[eorder]
    pad_row = row_base[node_win[n_of]] + j_of * 128 + node_slot[n_of]
    core_o = node_core[n_of]

    dt = ml_dtypes.bfloat16 if BF16 else np.float32
    edges_rows = np.zeros((N_CORES, rows, D), dt)
    edges_rows[core_o, pad_row] = ea[eorder].astype(dt)

    # device layout: [128 partitions(=slot), t_total*128] tile-major
    edges_dev = np.ascontiguousarray(
        edges_rows.reshape(N_CORES, t_total, 128, D)
        .transpose(0, 2, 1, 3)
        .reshape(N_CORES, 128, t_total * D)
    )

    # per-core transposed x table [128(feat), NODES_PER_CORE] in local order
    x_loc = np.zeros((N_CORES, NODES_PER_CORE, D), np.float32)
    nloc = node_win * 128 + node_slot
    x_loc[node_core, nloc] = x
    xt_loc = np.ascontiguousarray(x_loc.transpose(0, 2, 1).astype(dt))

    return dict(
        tiles=tuple(int(t) for t in tiles),
        edges_dev=edges_dev,
        xt_loc=xt_loc,
        node_core=node_core,
        nloc=nloc,
    )


# ----------------------------------------------------------------------------
# Device program
# ----------------------------------------------------------------------------

_KERNEL_CACHE = {}


def _build(tiles):
    import concourse.bass as bass
    import concourse.tile as tile
    from concourse import bacc, mybir

    f32 = mybir.dt.float32
    cdt = mybir.dt.bfloat16 if BF16 else mybir.dt.float32
    t_total = sum(tiles)

    nc = bacc.Bacc("TRN2", target_bir_lowering=False, debug=False,
                   num_devices=N_CORES)

    edges_d = nc.dram_tensor("edges", [128, t_total * D], cdt, kind="ExternalInput")
    xt_d = nc.dram_tensor("xt_loc", [128, NODES_PER_CORE], cdt, kind="ExternalInput")
    ident_d = nc.dram_tensor("ident", [128, 128], cdt, kind="ExternalInput")
    wst_d = nc.dram_tensor("w_s_top", [128, 128], cdt, kind="ExternalInput")
    wsb_d = nc.dram_tensor("w_s_bot", [128, 128], cdt, kind="ExternalInput")
    wh_d = nc.dram_tensor("w_h", [128, 128], cdt, kind="ExternalInput")
    we_d = nc.dram_tensor("w_e", [128, 128], cdt, kind="ExternalInput")
    bs_d = nc.dram_tensor("b_s", [128, 1], f32, kind="ExternalInput")
    bh_d = nc.dram_tensor("b_h", [128, 1], f32, kind="ExternalInput")
    be_d = nc.dram_tensor("b_e_row", [1, 128], cdt, kind="ExternalInput")
    ones_d = nc.dram_tensor("ones_row", [1, 128], cdt, kind="ExternalInput")
    out_d = nc.dram_tensor("out", [NODES_PER_CORE, D], f32, kind="ExternalOutput")

    Relu = mybir.ActivationFunctionType.Relu
    Sqrt = mybir.ActivationFunctionType.Sqrt
    MUL = mybir.AluOpType.mult
    ADD = mybir.AluOpType.add

    with tile.TileContext(nc) as tc:
        with (
            tc.tile_pool(name="const", bufs=1) as constp,
            tc.tile_pool(name="edges", bufs=4) as edgep,
            tc.tile_pool(name="acts", bufs=3) as actp,
            tc.tile_pool(name="small", bufs=4) as smallp,
            tc.tile_pool(name="outs", bufs=3) as outp,
            tc.tile_pool(name="psA", bufs=3, space="PSUM") as psA,
            tc.tile_pool(name="psM", bufs=4, space="PSUM") as psM,
        ):
            ident = constp.tile([128, 128], cdt)
            nc.scalar.dma_start(ident[:], ident_d[:])
            wst = constp.tile([128, 128], cdt)
            nc.scalar.dma_start(wst[:], wst_d[:])
            wsb = constp.tile([128, 128], cdt)
            nc.scalar.dma_start(wsb[:], wsb_d[:])
            wh = constp.tile([128, 128], cdt)
            nc.scalar.dma_start(wh[:], wh_d[:])
            we = constp.tile([128, 128], cdt)
            nc.scalar.dma_start(we[:], we_d[:])
            bs = constp.tile([128, 1], f32)
            nc.scalar.dma_start(bs[:], bs_d[:])
            bh = constp.tile([128, 1], f32)
            nc.scalar.dma_start(bh[:], bh_d[:])
            be = constp.tile([1, 128], cdt)
            nc.scalar.dma_start(be[:], be_d[:])
            ones = constp.tile([1, 128], cdt)
            nc.scalar.dma_start(ones[:], ones_d[:])
            xt = constp.tile([128, NODES_PER_CORE], cdt)
            nc.scalar.dma_start(xt[:], xt_d[:])

            col = 0
            for w in range(NW):
                T = tiles[w]
                c0 = col * 128

                ew = edgep.tile([128, T * 128], cdt, tag="ew")
                nc.sync.dma_start(ew[:], edges_d[:, c0:c0 + T * 128])

                # segment sum: aggT[f, s] += edge_tile[s, f]^T  (rhs = identity)
                ps_agg = psA.tile([128, 128], f32)
                for t in range(T):
                    nc.tensor.matmul(
                        ps_agg[:], ew[:, t * 128:(t + 1) * 128], ident[:],
                        start=(t == 0), stop=(t == T - 1))
                agg = actp.tile([128, 128], cdt, tag="agg")
                nc.vector.tensor_copy(agg[:], ps_agg[:])

                # MLP layer 1
                ps_h = psM.tile([128, 128], f32, tag="mlp")
                nc.tensor.matmul(ps_h[:], wst[:], xt[:, w * 128:(w + 1) * 128],
                                 start=True, stop=False)
                nc.tensor.matmul(ps_h[:], wsb[:], agg[:], start=False, stop=True)
                h1 = actp.tile([128, 128], cdt, tag="h1")
                nc.scalar.activation(h1[:], ps_h[:], Relu, bias=bs[:, 0:1])

                # layer 2
                ps_h2 = psM.tile([128, 128], f32, tag="mlp")
                nc.tensor.matmul(ps_h2[:], wh[:], h1[:], start=True, stop=True)
                h2 = actp.tile([128, 128], cdt, tag="h2")
                nc.scalar.activation(h2[:], ps_h2[:], Relu, bias=bh[:, 0:1])

                # output layer, node-major; b_e added via K=1 ones matmul
                ps_o = psM.tile([128, 128], f32, tag="mlp")
                nc.tensor.matmul(ps_o[:], h2[:], we[:], start=True, stop=False)
                nc.tensor.matmul(ps_o[:], ones[:], be[:], start=False, stop=True)

                # layernorm over free axis (fp32), stats straight from PSUM
                st = smallp.tile([128, 6], f32, tag="st")
                nc.vector.bn_stats(st[:], ps_o[:])
                mv = smallp.tile([128, 2], f32, tag="mv")
                nc.vector.bn_aggr(mv[:], st[:])
                ve = smallp.tile([128, 1], f32, tag="ve")
                nc.vector.tensor_scalar_add(ve[:], mv[:, 1:2], EPS)
                sd = smallp.tile([128, 1], f32, tag="sd")
                nc.scalar.activation(sd[:], ve[:], Sqrt, bias=0.0)
                istd = smallp.tile([128, 1], f32, tag="istd")
                nc.vector.reciprocal(istd[:], sd[:])
                nm = smallp.tile([128, 1], f32, tag="nm")
                nc.vector.tensor_scalar(nm[:], mv[:, 0:1], -1.0, istd[:, 0:1],
                                        MUL, MUL)
                o = outp.tile([128, 128], f32, tag="o")
                nc.vector.tensor_scalar(o[:], ps_o[:], istd[:, 0:1], nm[:, 0:1],
                                        MUL, ADD)
                nc.sync.dma_start(out_d[w * 128:(w + 1) * 128, :], o[:])

                col += T

    nc.compile()
    return nc


def _get_kernel(tiles):
    key = tuple(tiles)
    if key not in _KERNEL_CACHE:
        _KERNEL_CACHE[key] = _build(key)
    return _KERNEL_CACHE[key]


# ----------------------------------------------------------------------------
# Entry point
# ----------------------------------------------------------------------------

def kernel(x, edge_attr, receivers, senders, W_s, b_s, W_h, b_h, W_e, b_e):
    from concourse.bass_utils import run_bass_kernel_spmd

    x = np.asarray(x, dtype=np.float32)
    edge_attr_np = np.asarray(edge_attr)
    W_s = np.asarray(W_s, dtype=np.float32)
    b_s = np.asarray(b_s, dtype=np.float32)
    W_h = np.asarray(W_h, dtype=np.float32)
    b_h = np.asarray(b_h, dtype=np.float32)
    W_e = np.asarray(W_e, dtype=np.float32)
    b_e = np.asarray(b_e, dtype=np.float32)

    p = _pack(x, edge_attr_np, receivers)
    nc = _get_kernel(p["tiles"])

    dt = ml_dtypes.bfloat16 if BF16 else np.float32
    common = {
        "ident": np.eye(128, dtype=dt),
        "w_s_top": np.ascontiguousarray(W_s[:128, :]).astype(dt),
        "w_s_bot": np.ascontiguousarray(W_s[128:, :]).astype(dt),
        "w_h": W_h.astype(dt),
        "w_e": W_e.astype(dt),
        "b_s": b_s.reshape(128, 1),
        "b_h": b_h.reshape(128, 1),
        "b_e_row": b_e.reshape(1, 128).astype(dt),
        "ones_row": np.ones((1, 128), dtype=dt),
    }
    in_maps = []
    for c in range(N_CORES):
        m = dict(common)
        m["edges"] = p["edges_dev"][c]
        m["xt_loc"] = p["xt_loc"][c]
        in_maps.append(m)

    res = run_bass_kernel_spmd(nc, in_maps, list(range(N_CORES)))

    out_full = np.empty((N_NODES, D), np.float32)
    node_core = p["node_core"]
    nloc = p["nloc"]
    for c in range(N_CORES):
        mask = node_core == c
        out_full[mask] = res.results[c]["out"][nloc[mask]]

    return (out_full, edge_attr_np, np.asarray(receivers), np.asarray(senders))


# revision 14
# speedup vs baseline: 1.8180x; 1.1696x over previous
"""Trainium2 Bass kernel for GNN NodeBlock (segment_sum + MLP + layernorm).

Strategy (8 NeuronCores, SPMD single program):
  - Host: deal nodes to cores serpentine by degree (equal per-core edge
    counts), then within each core sort nodes by degree desc and cut into
    98 windows of 128 nodes ("slots").  Edges are laid out SLOT-ALIGNED:
    tile t of window w holds the t-th edge of every slot (one edge per
    partition), zero-padded.  Grouping equal-degree nodes per window makes
    the padding ~1.3%.
  - Device segment-sum needs NO one-hot and NO DVE work: for each tile,
        matmul(psum, lhsT=edge_tile[s, f], rhs=identity)  accumulates
        aggT[f, s] += edge_tile[s, f]^T
    into PSUM (feature-major window aggregate).
  - MLP in feature-major layout (x pre-transposed on host):
        h1 = relu(W_s_top^T @ x^T + W_s_bot^T @ aggT + b_s)
        h2 = relu(W_h^T @ h1 + b_h)
        out[n, f] = h2^T @ W_e + b_e      (node-major via lhsT=h2)
    then layernorm over f (free axis) with bn_stats/bn_aggr in fp32.
  - Segment-sum + MLP run in bf16 (fp32 PSUM accumulate); layernorm fp32.
  - Host: gather per-core outputs back to the global node order.
"""

import sys

if "/opt/trn_rl_repo" not in sys.path:
    sys.path.insert(0, "/opt/trn_rl_repo")

import numpy as np
import ml_dtypes

N_NODES = 100000
N_EDGES = 1600000
D = 128
N_CORES = 8
NW = 98                       # windows (128-node groups) per core
NODES_PER_CORE = NW * 128     # local node-table slots per core
EPS = 1e-5
BF16 = True                   # compute dtype for segment-sum + MLP


# ----------------------------------------------------------------------------
# Host-side packing
# ----------------------------------------------------------------------------

def _pack(x, edge_attr, receivers):
    rec = np.asarray(receivers).astype(np.int64)
    x = np.asarray(x, dtype=np.float32)
    ea = np.ascontiguousarray(np.asarray(edge_attr, dtype=np.float32))

    deg = np.bincount(rec, minlength=N_NODES)

    # nodes -> cores, serpentine by degree desc => equal per-core edge counts
    order = np.argsort(-deg, kind="stable")
    i = np.arange(N_NODES)
    blk, pos = i // N_CORES, i % N_CORES
    core_of_rank = np.where(blk % 2 == 0, pos, N_CORES - 1 - pos)
    node_core = np.empty(N_NODES, np.int64)
    node_core[order] = core_of_rank

    # within each core: degree-desc blocks of 128 = windows; rank in block = slot
    node_win = np.empty(N_NODES, np.int64)
    node_slot = np.empty(N_NODES, np.int64)
    deg_sorted_by_core = {}
    for c in range(N_CORES):
        nodes_c = order[core_of_rank == c]          # degree-desc
        j = np.arange(len(nodes_c))
        node_win[nodes_c] = j // 128
        node_slot[nodes_c] = j % 128
        deg_sorted_by_core[c] = deg[nodes_c]

    # shared per-window tile counts = max degree in window across cores
    tiles = np.zeros(NW, np.int64)
    for c in range(N_CORES):
        dc = deg_sorted_by_core[c]
        for w in range(NW):
            seg = dc[w * 128:(w + 1) * 128]
            if len(seg):
                tiles[w] = max(tiles[w], int(seg.max()))
    tiles = np.maximum(tiles, 1)
    t_total = int(tiles.sum())
    rows = t_total * 128

    # edge row position: window base + j*128 + slot, j = index among node's edges
    erec = rec
    eorder = np.argsort(erec, kind="stable")        # group edges by node
    starts = np.concatenate([[0], np.cumsum(deg)])[:-1]
    j_of = np.arange(N_EDGES) - starts[erec[eorder]]

    row_base = np.concatenate([[0], np.cumsum(tiles * 128)])[:-1]   # per window
    n_of = erec[eorder]
    pad_row = row_base[node_win[n_of]] + j_of * 128 + node_slot[n_of]
    core_o = node_core[n_of]

    dt = ml_dtypes.bfloat16 if BF16 else np.float32
    edges_rows = np.zeros((N_CORES, rows, D), dt)
    edges_rows[core_o, pad_row] = ea[eorder].astype(dt)

    # device layout: FEATURE-major tiles — [128 partitions(=feature),
    # t_total*128] where tile t cols hold slots 0..127.  The segment-sum
    # matmul is then lhsT=identity (stationary, loaded once), rhs=tile.
    edges_dev = np.ascontiguousarray(edges_rows.transpose(0, 2, 1))

    # per-core transposed x table [128(feat), NODES_PER_CORE] in local order
    x_loc = np.zeros((N_CORES, NODES_PER_CORE, D), np.float32)
    nloc = node_win * 128 + node_slot
    x_loc[node_core, nloc] = x
    xt_loc = np.ascontiguousarray(x_loc.transpose(0, 2, 1).astype(dt))

    return dict(
        tiles=tuple(int(t) for t in tiles),
        edges_dev=edges_dev,
        xt_loc=xt_loc,
        node_core=node_core,
        nloc=nloc,
    )


# ----------------------------------------------------------------------------
# Device program
# ----------------------------------------------------------------------------

_KERNEL_CACHE = {}


def _build(tiles):
    import concourse.bass as bass
    import concourse.tile as tile
    from concourse import bacc, mybir

    f32 = mybir.dt.float32
    cdt = mybir.dt.bfloat16 if BF16 else mybir.dt.float32
    t_total = sum(tiles)

    nc = bacc.Bacc("TRN2", target_bir_lowering=False, debug=False,
                   num_devices=N_CORES)

    edges_d = nc.dram_tensor("edges", [128, t_total * D], cdt, kind="ExternalInput")
    xt_d = nc.dram_tensor("xt_loc", [128, NODES_PER_CORE], cdt, kind="ExternalInput")
    ident_d = nc.dram_tensor("ident", [128, 128], cdt, kind="ExternalInput")
    wst_d = nc.dram_tensor("w_s_top", [128, 128], cdt, kind="ExternalInput")
    wsb_d = nc.dram_tensor("w_s_bot", [128, 128], cdt, kind="ExternalInput")
    wh_d = nc.dram_tensor("w_h", [128, 128], cdt, kind="ExternalInput")
    we_d = nc.dram_tensor("w_e", [128, 128], cdt, kind="ExternalInput")
    bs_d = nc.dram_tensor("b_s", [128, 1], f32, kind="ExternalInput")
    bh_d = nc.dram_tensor("b_h", [128, 1], f32, kind="ExternalInput")
    be_d = nc.dram_tensor("b_e_row", [1, 128], cdt, kind="ExternalInput")
    ones_d = nc.dram_tensor("ones_row", [1, 128], cdt, kind="ExternalInput")
    out_d = nc.dram_tensor("out", [NODES_PER_CORE, D], f32, kind="ExternalOutput")

    Relu = mybir.ActivationFunctionType.Relu
    Sqrt = mybir.ActivationFunctionType.Sqrt
    MUL = mybir.AluOpType.mult
    ADD = mybir.AluOpType.add

    GS = 4                                    # windows per group
    groups = [list(range(g, min(g + GS, NW))) for g in range(0, NW, GS)]
    col_base = [0] * NW
    c = 0
    for w in range(NW):
        col_base[w] = c
        c += tiles[w]

    with tile.TileContext(nc) as tc:
        with (
            tc.tile_pool(name="const", bufs=1) as constp,
            tc.tile_pool(name="edges", bufs=6) as edgep,
            tc.tile_pool(name="aggs", bufs=3) as aggp,
            tc.tile_pool(name="acts", bufs=3) as actp,
            tc.tile_pool(name="small", bufs=6) as smallp,
            tc.tile_pool(name="outs", bufs=4) as outp,
            tc.tile_pool(name="psA", bufs=3, space="PSUM") as psA,
            tc.tile_pool(name="psH", bufs=2, space="PSUM") as psH,
            tc.tile_pool(name="psO", bufs=3, space="PSUM") as psO,
        ):
            ident = constp.tile([128, 128], cdt)
            nc.scalar.dma_start(ident[:], ident_d[:])
            wst = constp.tile([128, 128], cdt)
            nc.scalar.dma_start(wst[:], wst_d[:])
            wsb = constp.tile([128, 128], cdt)
            nc.scalar.dma_start(wsb[:], wsb_d[:])
            wh = constp.tile([128, 128], cdt)
            nc.scalar.dma_start(wh[:], wh_d[:])
            we = constp.tile([128, 128], cdt)
            nc.scalar.dma_start(we[:], we_d[:])
            bs = constp.tile([128, 1], f32)
            nc.scalar.dma_start(bs[:], bs_d[:])
            bh = constp.tile([128, 1], f32)
            nc.scalar.dma_start(bh[:], bh_d[:])
            be = constp.tile([1, 128], cdt)
            nc.scalar.dma_start(be[:], be_d[:])
            ones = constp.tile([1, 128], cdt)
            nc.scalar.dma_start(ones[:], ones_d[:])
            xt = constp.tile([128, NODES_PER_CORE], cdt)
            nc.scalar.dma_start(xt[:], xt_d[:])

            agg_of = {}

            def emit_seg(gi):
                ws = groups[gi]
                agg = aggp.tile([128, GS * 128], cdt, tag="agg")
                agg_of[gi] = agg
                for i, w in enumerate(ws):
                    T = tiles[w]
                    c0 = col_base[w] * 128
                    ew = edgep.tile([128, T * 128], cdt, tag="ew")
                    nc.sync.dma_start(ew[:], edges_d[:, c0:c0 + T * 128])
                    # aggT[f, s] += tile_t[f, s]: lhsT=identity (stationary,
                    # loaded once), rhs = feature-major edge tile.
                    ps_agg = psA.tile([128, 128], f32)
                    for t in range(T):
                        nc.tensor.matmul(
                            ps_agg[:], ident[:], ew[:, t * 128:(t + 1) * 128],
                            start=(t == 0), stop=(t == T - 1))
                    nc.scalar.copy(agg[:, i * 128:(i + 1) * 128], ps_agg[:])

            def emit_mlp(gi):
                ws = groups[gi]
                gw = len(ws) * 128
                g0 = ws[0] * 128
                agg = agg_of.pop(gi)

                ps_h = psH.tile([128, GS * 128], f32, tag="h")
                nc.tensor.matmul(ps_h[:, :gw], wst[:], xt[:, g0:g0 + gw],
                                 start=True, stop=False)
                nc.tensor.matmul(ps_h[:, :gw], wsb[:], agg[:, :gw],
                                 start=False, stop=True)
                h1 = actp.tile([128, GS * 128], cdt, tag="h1")
                nc.scalar.activation(h1[:, :gw], ps_h[:, :gw], Relu,
                                     bias=bs[:, 0:1])

                ps_h2 = psH.tile([128, GS * 128], f32, tag="h")
                nc.tensor.matmul(ps_h2[:, :gw], wh[:], h1[:, :gw],
                                 start=True, stop=True)
                h2 = actp.tile([128, GS * 128], cdt, tag="h2")
                nc.scalar.activation(h2[:, :gw], ps_h2[:, :gw], Relu,
                                     bias=bh[:, 0:1])

                ps_os = []
                for i, w in enumerate(ws):
                    ps_o = psO.tile([128, 128], f32)
                    nc.tensor.matmul(ps_o[:], h2[:, i * 128:(i + 1) * 128],
                                     we[:], start=True, stop=False)
                    ps_os.append(ps_o)
                for i, w in enumerate(ws):
                    nc.tensor.matmul(ps_os[i][:], ones[:], be[:],
                                     start=False, stop=True)

                for i, w in enumerate(ws):
                    ps_o = ps_os[i]
                    st = smallp.tile([128, 6], f32, tag="st")
                    nc.vector.bn_stats(st[:], ps_o[:])
                    mv = smallp.tile([128, 2], f32, tag="mv")
                    nc.vector.bn_aggr(mv[:], st[:])
                    ve = smallp.tile([128, 1], f32, tag="ve")
                    nc.vector.tensor_scalar_add(ve[:], mv[:, 1:2], EPS)
                    sd = smallp.tile([128, 1], f32, tag="sd")
                    nc.scalar.activation(sd[:], ve[:], Sqrt, bias=0.0)
                    istd = smallp.tile([128, 1], f32, tag="istd")
                    nc.vector.reciprocal(istd[:], sd[:])
                    nm = smallp.tile([128, 1], f32, tag="nm")
                    nc.vector.tensor_scalar(nm[:], mv[:, 0:1], -1.0,
                                            istd[:, 0:1], MUL, MUL)
                    o = outp.tile([128, 128], f32, tag="o")
                    nc.vector.tensor_scalar(o[:], ps_o[:], istd[:, 0:1],
                                            nm[:, 0:1], MUL, ADD)
                    nc.sync.dma_start(out_d[w * 128:(w + 1) * 128, :], o[:])

            emit_seg(0)
            for gi in range(len(groups)):
                if gi + 1 < len(groups):
                    emit_seg(gi + 1)
                emit_mlp(gi)

    nc.compile()
    return nc


def _get_kernel(tiles):
    key = tuple(tiles)
    if key not in _KERNEL_CACHE:
        _KERNEL_CACHE[key] = _build(key)
    return _KERNEL_CACHE[key]


# ----------------------------------------------------------------------------
# Entry point
# ----------------------------------------------------------------------------

def kernel(x, edge_attr, receivers, senders, W_s, b_s, W_h, b_h, W_e, b_e):
    from concourse.bass_utils import run_bass_kernel_spmd

    x = np.asarray(x, dtype=np.float32)
    edge_attr_np = np.asarray(edge_attr)
    W_s = np.asarray(W_s, dtype=np.float32)
    b_s = np.asarray(b_s, dtype=np.float32)
    W_h = np.asarray(W_h, dtype=np.float32)
    b_h = np.asarray(b_h, dtype=np.float32)
    W_e = np.asarray(W_e, dtype=np.float32)
    b_e = np.asarray(b_e, dtype=np.float32)

    p = _pack(x, edge_attr_np, receivers)
    nc = _get_kernel(p["tiles"])

    dt = ml_dtypes.bfloat16 if BF16 else np.float32
    common = {
        "ident": np.eye(128, dtype=dt),
        "w_s_top": np.ascontiguousarray(W_s[:128, :]).astype(dt),
        "w_s_bot": np.ascontiguousarray(W_s[128:, :]).astype(dt),
        "w_h": W_h.astype(dt),
        "w_e": W_e.astype(dt),
        "b_s": b_s.reshape(128, 1),
        "b_h": b_h.reshape(128, 1),
        "b_e_row": b_e.reshape(1, 128).astype(dt),
        "ones_row": np.ones((1, 128), dtype=dt),
    }
    in_maps = []
    for c in range(N_CORES):
        m = dict(common)
        m["edges"] = p["edges_dev"][c]
        m["xt_loc"] = p["xt_loc"][c]
        in_maps.append(m)

    res = run_bass_kernel_spmd(nc, in_maps, list(range(N_CORES)))

    out_full = np.empty((N_NODES, D), np.float32)
    node_core = p["node_core"]
    nloc = p["nloc"]
    for c in range(N_CORES):
        mask = node_core == c
        out_full[mask] = res.results[c]["out"][nloc[mask]]

    return (out_full, edge_attr_np, np.asarray(receivers), np.asarray(senders))
